# revision 1
# baseline (speedup 1.0000x reference)
"""CrossCovarianceAttn Trainium2 kernel.

Data-parallel over B=8 across 8 NeuronCores; each core runs the full model on
one batch element. All big matmuls run in fp32r (tf32-class precision, 1
cyc/row for moving dim >= 256); PE transposes run in fp32 (exact). Norms over
the token dim come from Gram-matrix diagonals computed on the PE (no
partition reductions); covariance C_h, Gq_h, Gk_h come from two fused
matmuls per head against the head-interleaved [q_h|k_h] block. DMA traffic
is split across both hardware DGE queues (SP + ACT).

Pipeline per core:
  phase 0: transpose w_qkv -> w_qkT (q|k head-interleaved columns) + w_vT
  phase 1: per 512-token tile: PE-transpose x -> xT; qk = xT.T @ w_qkT
           (token-major); vT = w_vT.T @ xT (feature-major) -> DRAM;
           covariance+Gram accumulated in PSUM, flushed per tile
  phase 2: Gram diagonals -> 1/max(||.||, eps); batched all-head softmax
           (free-dim broadcast ops); transpose attn
  phase 3 (sw-pipelined): attn_h @ vT_h -> outT (head-major);
           y = outT.T @ w_projT + b -> out   (contraction in 96-row blocks)
"""
import os
import sys

sys.path.insert(0, "/opt/trn_rl_repo")

import numpy as np

import concourse.bass as bass
import concourse.mybir as mybir
import concourse.tile as tile
from concourse import bacc
from concourse.bass_utils import run_bass_kernel_spmd
from concourse.masks import make_identity

FP32 = mybir.dt.float32
FP32R = mybir.dt.float32r
BF16 = mybir.dt.bfloat16

N_TOK = 4096
C = 768
H = 8
HD = 96
C3 = 3 * C
TOK_TILE = 512
N_TILES = N_TOK // TOK_TILE
CHUNKS = TOK_TILE // 128
KK = C // 128
EPS = 1e-12

_CACHED_NC = None
XTP_BUFS = int(os.environ.get("XTP_BUFS", "2"))
QKP_BUFS = int(os.environ.get("QKP_BUFS", "2"))
PSTR_BUFS = int(os.environ.get("PSTR_BUFS", "2"))
PSMM_BUFS = int(os.environ.get("PSMM_BUFS", "6"))
XCOPY_ACT = os.environ.get("XCOPY_ACT", "0") == "1"


def _qk_perm_strips(m):
    """Strips (j_start, length, dst_col) mapping w_qkv c3-block m's local row
    j to head-interleaved columns: q row (96h+d) -> 192h+d, k -> 192h+96+d."""
    strips = []
    j = 0
    while j < 128:
        c3 = 128 * m + j
        if c3 < C:
            h, d = divmod(c3, HD)
            dst = 192 * h + d
            run = min(128 - j, HD - d)
        else:
            h, d = divmod(c3 - C, HD)
            dst = 192 * h + HD + d
            run = min(128 - j, HD - d)
        strips.append((j, run, dst))
        j += run
    return strips


def build_nc():
    nc = bacc.Bacc("TRN2", target_bir_lowering=False, debug=False, num_devices=8)

    x_d = nc.dram_tensor("x", (N_TOK, C), FP32, kind="ExternalInput").ap()
    wqkv_d = nc.dram_tensor("w_qkv", (C3, C), FP32, kind="ExternalInput").ap()
    temp_d = nc.dram_tensor("temperature", (H, 1, 1), FP32, kind="ExternalInput").ap()
    wproj_d = nc.dram_tensor("w_proj", (C, C), FP32, kind="ExternalInput").ap()
    bproj_d = nc.dram_tensor("b_proj", (C,), FP32, kind="ExternalInput").ap()
    out_d = nc.dram_tensor("out", (N_TOK, C), FP32, kind="ExternalOutput").ap()

    with tile.TileContext(nc) as tc:
        _build(tc, nc, x_d, wqkv_d, temp_d, wproj_d, bproj_d, out_d)
    nc.compile()
    return nc


def _build(tc, nc, x_d, wqkv_d, temp_d, wproj_d, bproj_d, out_d):
    import contextlib

    ctx = contextlib.ExitStack()
    with ctx:
        singles = ctx.enter_context(tc.tile_pool(name="singles", bufs=1))
        dram = ctx.enter_context(tc.tile_pool(name="dram", bufs=1, space="DRAM"))
        ps_tr = ctx.enter_context(tc.tile_pool(name="ps_tr", bufs=PSTR_BUFS, space="PSUM"))

        ident = singles.tile([128, 128], FP32)
        make_identity(nc, ident)

        b_all = singles.tile([128, C], FP32)
        nc.gpsimd.dma_start(
            b_all, bass.AP(tensor=bproj_d.tensor, offset=bproj_d.offset,
                           ap=[[0, 128], [1, C]]))
        temp_all = singles.tile([HD, H], FP32)
        nc.gpsimd.dma_start(
            temp_all, bass.AP(tensor=temp_d.tensor, offset=temp_d.offset,
                              ap=[[0, HD], [1, H]]))

        warm = singles.tile([1, 1], FP32)
        nc.vector.memset(warm, 0.5)
        nc.scalar.activation(warm, warm, mybir.ActivationFunctionType.Exp)
        nc.scalar.sqrt(warm, warm)

        cg_accum = singles.tile([HD, H, 384], FP32)
        nc.vector.memset(cg_accum, 0.0)
        attnT = singles.tile([HD, H, HD], FP32R)

        vT_dram = dram.tile([C, N_TOK], FP32R)

        # ---------------- phase 0: qkv weight prep ----------------
        with tc.tile_pool(name="wload", bufs=2) as wload, \
             tc.tile_pool(name="wqk_pool", bufs=1) as wqk_pool:
            w_qkT = wqk_pool.tile([128, KK, 2 * C], FP32R)
            w_vT = wqk_pool.tile([128, KK, C], FP32R)

            for m in range(C3 // 128):
                w_blk = wload.tile([128, C], FP32, name="w_blk")
                nc.sync.dma_start(w_blk, wqkv_d[m * 128:(m + 1) * 128, :])
                for kk in range(KK):
                    tps = ps_tr.tile([128, 128], FP32, name="tps", tag="tr")
                    nc.tensor.transpose(tps, w_blk[:, kk * 128:(kk + 1) * 128],
                                        ident)
                    if m < 12:
                        for (j0, run, dst) in _qk_perm_strips(m):
                            nc.vector.tensor_copy(
                                w_qkT[:, kk, dst:dst + run], tps[:, j0:j0 + run])
                    else:
                        base = m * 128 - 2 * C
                        nc.scalar.copy(w_vT[:, kk, base:base + 128], tps)

            # ---------------- phase 1 ----------------
            with tc.tile_pool(name="xin", bufs=8) as xin, \
                 tc.tile_pool(name="xtp", bufs=XTP_BUFS) as xtp, \
                 tc.tile_pool(name="qkp", bufs=QKP_BUFS) as qkp, \
                 tc.tile_pool(name="vtsb", bufs=2) as vtsb, \
                 tc.tile_pool(name="ps_mm", bufs=PSMM_BUFS, space="PSUM") as ps_mm:
                for t in range(N_TILES):
                    t0 = t * TOK_TILE
                    xT_t = xtp.tile([128, KK, TOK_TILE], FP32R, name="xT_t")
                    qk_t = qkp.tile([128, CHUNKS, 1536], BF16, name="qk_t")

                    xcs = []
                    for c in range(CHUNKS):
                        x_c = xin.tile([128, C], FP32, name="x_c")
                        nc.sync.dma_start(
                            x_c, x_d[t0 + c * 128: t0 + (c + 1) * 128, :])
                        xcs.append(x_c)
                    for kk in range(KK):
                        xps = ps_tr.tile([128, TOK_TILE], FP32, name="xps",
                                         tag="tr")
                        for c in range(CHUNKS):
                            nc.tensor.transpose(
                                xps[:, c * 128:(c + 1) * 128],
                                xcs[c][:, kk * 128:(kk + 1) * 128], ident)
                        if XCOPY_ACT and kk % 2 == 1:
                            nc.scalar.copy(xT_t[:, kk, :], xps)
                        else:
                            nc.vector.tensor_copy(xT_t[:, kk, :], xps)

                    # qk = xT.T @ w_qkT (token-major, head-interleaved cols).
                    # fp32r matmuls are always self-loading, so piece-outer
                    # order costs nothing and psums rotate one at a time.
                    for c in range(CHUNKS):
                        for p in range(3):
                            mmps = ps_mm.tile([128, 512], FP32, name="mmps",
                                              tag="s")
                            for kk in range(KK):
                                nc.tensor.matmul(
                                    mmps, xT_t[:, kk, c * 128:(c + 1) * 128],
                                    w_qkT[:, kk, p * 512:(p + 1) * 512],
                                    start=(kk == 0), stop=(kk == KK - 1))
                            if p == 1:
                                nc.scalar.copy(
                                    qk_t[:, c, p * 512:(p + 1) * 512], mmps)
                            else:
                                nc.vector.tensor_copy(
                                    qk_t[:, c, p * 512:(p + 1) * 512], mmps)

                    # vT = w_vT.T @ xT (feature-major) -> DRAM
                    vt_sb = vtsb.tile([128, KK, TOK_TILE], FP32R, name="vt_sb")
                    for m in range(KK):
                        vps = ps_mm.tile([128, TOK_TILE], FP32, name="vps",
                                         tag="s")
                        for kk in range(KK):
                            nc.tensor.matmul(
                                vps, w_vT[:, kk, m * 128:(m + 1) * 128],
                                xT_t[:, kk, :],
                                start=(kk == 0), stop=(kk == KK - 1))
                        nc.scalar.copy(vt_sb[:, m, :], vps)
                    nc.scalar.dma_start(
                        vT_dram[:, t0:t0 + TOK_TILE].rearrange(
                            "(s p) n -> p s n", p=128),
                        vt_sb)

                    # covariance + Gram: one psum bank per head, single
                    # accumulation group (one start, one stop)
                    for h in range(H):
                        cg_ps = ps_mm.tile([HD, 384], FP32, name="cg_ps",
                                           tag="s")
                        for c in range(CHUNKS):
                            rhs = qk_t[:, c, 192 * h:192 * h + 192]
                            nc.tensor.matmul(
                                cg_ps[:, 0:192],
                                qk_t[:, c, 192 * h:192 * h + HD], rhs,
                                start=(c == 0), stop=False)
                            nc.tensor.matmul(
                                cg_ps[:, 192:384],
                                qk_t[:, c, 192 * h + HD:192 * h + 192], rhs,
                                start=False, stop=(c == CHUNKS - 1))
                        nc.vector.tensor_add(
                            cg_accum[:, h, :], cg_ps, cg_accum[:, h, :])

        # ---------------- phase 3 pools; w_projT prep emitted first so the
        # PE has work while the DVE/ACT-heavy phase 2 chain runs ----------
        with tc.tile_pool(name="wpp", bufs=1) as wpp, \
             tc.tile_pool(name="wpload", bufs=2) as wpload, \
             tc.tile_pool(name="vtp", bufs=2) as vtp, \
             tc.tile_pool(name="otp", bufs=2) as otp, \
             tc.tile_pool(name="yp", bufs=2) as yp, \
             tc.tile_pool(name="ps_o", bufs=2, space="PSUM") as ps_o, \
             tc.tile_pool(name="ps_y", bufs=4, space="PSUM") as ps_y:
            # w_proj (cout, c) -> w_projT (96 c-rows per head, cout free)
            w_projT = wpp.tile([HD, H, C], FP32R)
            for n in range(KK):
                wp_blk = wpload.tile([128, C], FP32, name="wp_blk")
                nc.sync.dma_start(wp_blk, wproj_d[n * 128:(n + 1) * 128, :])
                for h in range(H):
                    tps2 = ps_tr.tile([HD, 128], FP32, name="tps2", tag="tr")
                    nc.tensor.transpose(
                        tps2, wp_blk[:, h * HD:(h + 1) * HD], ident)
                    nc.vector.tensor_copy(
                        w_projT[:, h, n * 128:(n + 1) * 128], tps2)

            # ---------------- phase 2: norms + softmax ----------------
            # cg_accum[:, h, :]: [0:96] Gq, [96:192] C, [352:448] Gk
            ident96 = ident[0:96, 0:96]
            identb = ident96[:, None, :].to_broadcast((HD, H, HD))
            sq = singles.tile([HD, 2, H], FP32)
            scr = singles.tile([HD, H, HD], FP32)
            nc.vector.tensor_tensor(
                scr, cg_accum[:, :, 0:HD], identb, mybir.AluOpType.mult)
            nc.vector.reduce_sum(
                sq[:, 0, :, None], scr, axis=mybir.AxisListType.X)
            nc.vector.tensor_tensor(
                scr, cg_accum[:, :, 288:384], identb, mybir.AluOpType.mult)
            nc.vector.reduce_sum(
                sq[:, 1, :, None], scr, axis=mybir.AxisListType.X)

            nrm = singles.tile([HD, 2, H], FP32)
            nc.scalar.sqrt(nrm, sq)
            nc.vector.tensor_scalar_max(nrm, nrm, EPS)
            rnorm = singles.tile([HD, 2, H], FP32)
            nc.vector.reciprocal(rnorm, nrm)
            rq = singles.tile([HD, H], FP32)
            nc.vector.tensor_tensor(rq, rnorm[:, 0, :], temp_all,
                                    mybir.AluOpType.mult)

            # rk to the free dim: store h-major to DRAM, broadcast-read back
            rk_scr = dram.tile([H, HD], FP32)
            nc.sync.dma_start(
                bass.AP(tensor=rk_scr.tensor, offset=rk_scr.offset,
                        ap=[[1, HD], [HD, H]]),
                rnorm[:, 1, :])
            rk_all = singles.tile([HD, H, HD], FP32)
            nc.sync.dma_start(
                rk_all, bass.AP(tensor=rk_scr.tensor, offset=rk_scr.offset,
                                ap=[[0, HD], [1, H * HD]]))

            # batched all-head softmax: logits = C * rq[d] * rk[e] * temp
            attL = singles.tile([HD, H, HD], FP32)
            nc.vector.tensor_tensor(
                attL, cg_accum[:, :, HD:2 * HD],
                rq[:, :, None].to_broadcast((HD, H, HD)), mybir.AluOpType.mult)
            nc.vector.tensor_tensor(attL, attL, rk_all, mybir.AluOpType.mult)
            mxa = singles.tile([HD, H, 1], FP32)
            nc.vector.reduce_max(mxa, attL, axis=mybir.AxisListType.X)
            nc.vector.tensor_tensor(
                attL, attL, mxa.to_broadcast((HD, H, HD)),
                mybir.AluOpType.subtract)
            nc.scalar.activation(attL, attL, mybir.ActivationFunctionType.Exp)
            sea = singles.tile([HD, H, 1], FP32)
            nc.vector.reduce_sum(sea, attL, axis=mybir.AxisListType.X)
            rsea = singles.tile([HD, H, 1], FP32)
            nc.vector.reciprocal(rsea, sea)
            nc.vector.tensor_tensor(
                attL, attL, rsea.to_broadcast((HD, H, HD)),
                mybir.AluOpType.mult)
            for h in range(H):
                atps = ps_tr.tile([HD, HD], FP32, name="atps", tag="tr")
                nc.tensor.transpose(atps, attL[:, h, :], ident96)
                nc.vector.tensor_copy(attnT[:, h, :], atps)

            # ---------------- phase 3: attn@v + proj, sw-pipelined --------
            def attnv_stage(t):
                t0 = t * TOK_TILE
                vT_t = vtp.tile([HD, H, TOK_TILE], FP32R, name="vT_t")
                nc.scalar.dma_start(
                    vT_t,
                    vT_dram[:, t0:t0 + TOK_TILE].rearrange(
                        "(h d) n -> d h n", h=H))
                otsb = otp.tile([HD, H, TOK_TILE], FP32R, name="otsb")
                for h in range(H):
                    ops_ = ps_o.tile([HD, TOK_TILE], FP32, name="ops_")
                    nc.tensor.matmul(ops_, attnT[:, h, :], vT_t[:, h, :],
                                     start=True, stop=True)
                    if h % 2 == 0:
                        nc.vector.tensor_copy(otsb[:, h, :], ops_)
                    else:
                        nc.scalar.copy(otsb[:, h, :], ops_)
                return otsb

            def proj_stage(t, otsb):
                t0 = t * TOK_TILE
                y_t = yp.tile([128, CHUNKS, C], FP32, name="y_t")
                for c in range(CHUNKS):
                    for (off, width) in ((0, 512), (512, 256)):
                        yps = ps_y.tile([128, 512], FP32, name="yps")
                        for h in range(H):
                            nc.tensor.matmul(
                                yps[:, :width],
                                otsb[:, h, c * 128:(c + 1) * 128],
                                w_projT[:, h, off:off + width],
                                start=(h == 0), stop=(h == H - 1))
                        nc.vector.tensor_tensor(
                            y_t[:, c, off:off + width], yps[:, :width],
                            b_all[:, off:off + width], mybir.AluOpType.add)
                nc.sync.dma_start(
                    out_d[t0:t0 + TOK_TILE, :].rearrange(
                        "(c p) f -> p c f", p=128),
                    y_t)

            pend = None
            for t in range(N_TILES):
                cur = attnv_stage(t)
                if pend is not None:
                    proj_stage(*pend)
                pend = (t, cur)
            proj_stage(*pend)


def _get_nc():
    global _CACHED_NC
    if _CACHED_NC is None:
        _CACHED_NC = build_nc()
    return _CACHED_NC


def kernel(x, w_qkv, temperature, w_proj, b_proj):
    nc = _get_nc()
    x = np.ascontiguousarray(np.asarray(x, dtype=np.float32))
    in_maps = []
    for b in range(8):
        in_maps.append({
            "x": x[b],
            "w_qkv": np.asarray(w_qkv, dtype=np.float32),
            "temperature": np.asarray(temperature, dtype=np.float32),
            "w_proj": np.asarray(w_proj, dtype=np.float32),
            "b_proj": np.asarray(b_proj, dtype=np.float32),
        })
    res = run_bass_kernel_spmd(nc, in_maps, core_ids=list(range(8)))
    return np.stack([r["out"] for r in res.results], axis=0)



# revision 23
# speedup vs baseline: 1.7004x; 1.7004x over previous
"""CrossCovarianceAttn Trainium2 kernel.

Data-parallel over B=8 across 8 NeuronCores; each core runs the full model on
one batch element.

Numerics: the q/k projection and the per-head covariance/Gram matmuls run in
fp8e4m3 with perf_mode=DoubleRow (two 128-row k-tiles per instruction, 0.5
cyc/row) — safe because q,k are l2-normalized over the token dim downstream,
which cancels the fp8 scaling exactly and the softmax logits are small
(|logit| <= temperature by Cauchy-Schwarz, which also lets softmax skip the
max-subtraction). The v projection and attn@v run in bf16; the output
projection runs fp32r on a dense 128-row contraction. PE transposes run fp32r
(1.5 cyc/row; tf32-ish truncation is harmless, everything transposed feeds
bf16/fp8 or is attn in [0,1]).

Cross-partition relayouts use DMA (compute engines cannot cross partitions):
  - vT (c-major [128,6,N] bf16 from the PE) -> vt_all [96,H,N] per-head
    layout via sbuf->sbuf strip DMAs.
  - attn@v psums ([96,512] per head) -> otsb128 [128,6,512] fp32r via
    gpsimd psum->sbuf strip DMAs (fp32->fp32r is a bit-identical
    reinterpret, the only cast SWDGE can do), so the projection contracts
    dense 128-row blocks instead of 96-row head blocks.

Pipeline per core:
  phase 0: w_qkv -> w_qk_f8 (x64, plain layout, 3 per-512-col tiles so the
           first qk matmul only waits on a third of the prep) + w_vT bf16
  phase 1: per 512-token tile: PE-transpose x -> xT (fp32r); cast to xT_bf
           (DVE/ACT) and xT_bf -> xT_f8 on GpSimd (x32); qk = xT_f8.T @
           w_qk_f8 (DoubleRow) -> qk_t fp8 (x 26/1135); vT = w_vT.T @ xT_bf
           -> strip-DMA into vt_all; covariance+Gram (3 DoubleRow matmuls
           per head: Gq | C | Gk) accumulated over pairs of tiles, flushed
           into cg_accum fp32
  phase 2: Gram diagonals -> 1/max(||.||,eps); batched all-head softmax
           (no max-sub); transpose attn -> attnT bf16
  phase 3 (sw-pipelined): attn_h @ vt_all_h -> otsb128 strips;
           y = otsb128.T @ w_projT128 + b -> out
"""
import os
import sys

sys.path.insert(0, "/opt/trn_rl_repo")

import numpy as np

import concourse.bass as bass
import concourse.mybir as mybir
import concourse.tile as tile
from concourse import bacc
from concourse.bass_utils import run_bass_kernel_spmd
from concourse.masks import make_identity

FP32 = mybir.dt.float32
FP32R = mybir.dt.float32r
BF16 = mybir.dt.bfloat16
FP8 = mybir.dt.float8e4
DR = mybir.MatmulPerfMode.DoubleRow

N_TOK = 4096
C = 768
H = 8
HD = 96
C3 = 3 * C
TOK_TILE = 512
N_TILES = N_TOK // TOK_TILE
CHUNKS = TOK_TILE // 128
KK = C // 128
EPS = 1e-12

S_X = 32.0            # x -> fp8 scale
S_W = 64.0            # w_q/w_k -> fp8 scale
C_QK = 26.0 / 1135.0  # qk psum (scaled x2048) -> fp8 scale

_CACHED_NC = None
XTP_BUFS = int(os.environ.get("XTP_BUFS", "2"))
QKP_BUFS = int(os.environ.get("QKP_BUFS", "3"))
PSTR_BUFS = int(os.environ.get("PSTR_BUFS", "2"))
PSMM_BUFS = int(os.environ.get("PSMM_BUFS", "6"))
XIN_BUFS = int(os.environ.get("XIN_BUFS", "2"))


def _vt_strips():
    """Strips (m, p0, run, h, d0): vt_sb partition p of block m holds
    v-feature c = 128m + p = 96h + d -> vt_all partition d, head h."""
    strips = []
    for m in range(KK):
        c0 = 128 * m
        p = 0
        while p < 128:
            h, d = divmod(c0 + p, HD)
            run = min(128 - p, HD - d)
            strips.append((m, p, run, h, d))
            p += run
    return strips


def _ot_strips():
    """Strips (h, d0, run, m, p0): attnv psum for head h, row d holds
    out-feature c = 96h + d -> otsb128 partition p = c % 128, block m."""
    strips = []
    for h in range(H):
        c0 = HD * h
        d = 0
        while d < HD:
            m, p = divmod(c0 + d, 128)
            run = min(HD - d, 128 - p)
            strips.append((h, d, run, m, p))
            d += run
    return strips


def phase2(nc, tc, singles, dram, ps_tr, cg_accum, attnT, temp_all, ident96f):
    """Norms + batched all-head softmax -> attnT (bf16).

    cg_accum[:, h, :]: [0:96] Gq, [96:192] Gk, [192:288] C. The Gq|Gk
    adjacency lets one mask-mult + one reduce grab both sets of diagonals.
    |logit| <= temperature (Cauchy-Schwarz on unit vectors), so exp runs
    directly with no max-subtraction.
    """
    import concourse.mybir as mybir

    identb = ident96f[:, None, None, :].to_broadcast((HD, H, 2, HD))
    sq = singles.tile([HD, H, 2], FP32)
    scr = singles.tile([HD, H, 2, HD], FP32)
    nc.vector.tensor_tensor(
        scr, cg_accum[:, :, 0:2 * HD].rearrange(
            "d h (two e) -> d h two e", two=2),
        identb, mybir.AluOpType.mult)
    nc.vector.reduce_sum(sq[:, :, :, None], scr, axis=mybir.AxisListType.X)

    nrm = singles.tile([HD, H, 2], FP32)
    nc.scalar.sqrt(nrm, sq)
    nc.vector.tensor_scalar_max(nrm, nrm, EPS)
    rnorm = singles.tile([HD, H, 2], FP32)
    nc.vector.reciprocal(rnorm, nrm)
    rq = singles.tile([HD, H], FP32)
    nc.vector.tensor_tensor(rq, rnorm[:, :, 0], temp_all,
                            mybir.AluOpType.mult)

    # rk to the free dim: store h-major to DRAM, broadcast-read back
    rk_scr = dram.tile([H, HD], FP32)
    nc.sync.dma_start(
        bass.AP(tensor=rk_scr.tensor, offset=rk_scr.offset,
                ap=[[1, HD], [HD, H]]),
        rnorm[:, :, 1])
    rk_all = singles.tile([HD, H, HD], FP32)
    nc.sync.dma_start(
        rk_all, bass.AP(tensor=rk_scr.tensor, offset=rk_scr.offset,
                        ap=[[0, HD], [1, H * HD]]))

    attL = singles.tile([HD, H, HD], FP32)
    nc.vector.tensor_tensor(
        attL, cg_accum[:, :, 2 * HD:3 * HD],
        rq[:, :, None].to_broadcast((HD, H, HD)), mybir.AluOpType.mult)
    nc.vector.tensor_tensor(attL, attL, rk_all, mybir.AluOpType.mult)
    nc.scalar.activation(attL, attL, mybir.ActivationFunctionType.Exp)
    sea = singles.tile([HD, H, 1], FP32)
    nc.vector.reduce_sum(sea, attL, axis=mybir.AxisListType.X)
    rsea = singles.tile([HD, H, 1], FP32)
    nc.vector.reciprocal(rsea, sea)
    nc.vector.tensor_tensor(
        attL, attL, rsea.to_broadcast((HD, H, HD)), mybir.AluOpType.mult)
    for h in range(H):
        atps = ps_tr.tile([HD, HD], FP32, name="atps", tag="tr")
        nc.tensor.transpose(atps, attL[:, h, :], ident96f)
        if h % 2 == 0:
            nc.vector.tensor_copy(attnT[:, h, :], atps)
        else:
            nc.scalar.copy(attnT[:, h, :], atps)


def build_nc():
    nc = bacc.Bacc("TRN2", target_bir_lowering=False, debug=False, num_devices=8)

    x_d = nc.dram_tensor("x", (N_TOK, C), FP32R, kind="ExternalInput").ap()
    wqkv_d = nc.dram_tensor("w_qkv", (C3, C), FP32R, kind="ExternalInput").ap()
    temp_d = nc.dram_tensor("temperature", (H, 1, 1), FP32, kind="ExternalInput").ap()
    wproj_d = nc.dram_tensor("w_proj", (C, C), FP32R, kind="ExternalInput").ap()
    bproj_d = nc.dram_tensor("b_proj", (C,), FP32, kind="ExternalInput").ap()
    out_d = nc.dram_tensor("out", (N_TOK, C), FP32, kind="ExternalOutput").ap()

    with tile.TileContext(nc) as tc:
        _build(tc, nc, x_d, wqkv_d, temp_d, wproj_d, bproj_d, out_d)
    nc.compile()
    return nc


def _build(tc, nc, x_d, wqkv_d, temp_d, wproj_d, bproj_d, out_d):
    import contextlib

    ctx = contextlib.ExitStack()
    with ctx:
        singles = ctx.enter_context(tc.tile_pool(name="singles", bufs=1))
        dram = ctx.enter_context(tc.tile_pool(name="dram", bufs=1, space="DRAM"))
        ps_tr = ctx.enter_context(tc.tile_pool(name="ps_tr", bufs=PSTR_BUFS, space="PSUM"))

        ident_f32 = singles.tile([128, 128], FP32)
        make_identity(nc, ident_f32)
        ident = singles.tile([128, 128], FP32R)
        nc.vector.tensor_copy(ident, ident_f32)
        ident96f = ident_f32[0:HD, 0:HD]

        b_all = singles.tile([128, C], FP32)
        nc.gpsimd.dma_start(
            b_all, bass.AP(tensor=bproj_d.tensor, offset=bproj_d.offset,
                           ap=[[0, 128], [1, C]]))
        temp_all = singles.tile([HD, H], FP32)
        nc.gpsimd.dma_start(
            temp_all, bass.AP(tensor=temp_d.tensor, offset=temp_d.offset,
                              ap=[[0, HD], [1, H]]))

        warm = singles.tile([1, 1], FP32)
        nc.vector.memset(warm, 0.5)
        nc.scalar.activation(warm, warm, mybir.ActivationFunctionType.Exp)
        nc.scalar.sqrt(warm, warm)

        cg_accum = singles.tile([HD, H, 288], FP32)
        nc.vector.memset(cg_accum, 0.0)
        attnT = singles.tile([HD, H, HD], BF16)
        vt_all = singles.tile([HD, H, N_TOK], BF16)

        # ---------------- phase 0 + 1 ----------------
        # w_qk plain layout: w_qk_f8[p][:, kk, j] = 64 * w_qkv[512p+j, 128kk+:]
        # (cols 0..767 across the 3 tiles = q rows, 768..1535 = k rows)
        with tc.tile_pool(name="wqk_pool", bufs=1) as wqk_pool, \
             tc.tile_pool(name="xin", bufs=XIN_BUFS) as xin, \
             tc.tile_pool(name="xtp", bufs=XTP_BUFS) as xtp, \
             tc.tile_pool(name="qkp", bufs=QKP_BUFS) as qkp, \
             tc.tile_pool(name="vtsb", bufs=2) as vtsb, \
             tc.tile_pool(name="ps_mm", bufs=PSMM_BUFS, space="PSUM") as ps_mm:
            w_qk_f8 = [wqk_pool.tile([128, KK, 512], FP8, name=f"wqk{p}")
                       for p in range(3)]
            w_vT = wqk_pool.tile([128, KK, C], BF16)
            vstrips = _vt_strips()
            state = {"vt_sb": None, "qk_pair": []}

            def xstage(t):
                """x load + PE transpose + bf16/fp8 casts for one tile."""
                t0 = t * TOK_TILE
                x_t = xin.tile([128, CHUNKS, C], FP32R, name="x_t")
                nc.sync.dma_start(
                    x_t, x_d[t0:t0 + TOK_TILE, :].rearrange(
                        "(c p) f -> p c f", p=128))
                xT_bf = xtp.tile([128, KK, TOK_TILE], BF16, name="xT_bf")
                xT_f8 = xtp.tile([128, KK, TOK_TILE], FP8, name="xT_f8")
                for kk in range(KK):
                    xps = ps_tr.tile([128, TOK_TILE], FP32R, name="xps",
                                     tag="tr")
                    for c in range(CHUNKS):
                        nc.tensor.transpose(
                            xps[:, c * 128:(c + 1) * 128],
                            x_t[:, c, kk * 128:(kk + 1) * 128], ident)
                    if kk % 2 == 0:
                        nc.vector.tensor_copy(xT_bf[:, kk, :], xps)
                    else:
                        nc.scalar.copy(xT_bf[:, kk, :], xps)
                    nc.gpsimd.tensor_scalar_mul(
                        xT_f8[:, kk, :], xT_bf[:, kk, :], S_X)
                return xT_bf, xT_f8

            def mmstage(t, xT_bf, xT_f8):
                """qk + vT matmuls, vt strips, covariance for one tile."""
                t0 = t * TOK_TILE
                qk_t = qkp.tile([128, CHUNKS, 1536], FP8, name="qk_t")

                # qk = xT.T @ w_qk (token-major; fp8 DoubleRow pairs)
                for c in range(CHUNKS):
                    for p in range(3):
                        mmps = ps_mm.tile([128, 512], FP32, name="mmps",
                                          tag="s")
                        for i in range(KK // 2):
                            nc.tensor.matmul(
                                mmps,
                                xT_f8[:, 2 * i:2 * i + 2,
                                      c * 128:(c + 1) * 128],
                                w_qk_f8[p][:, 2 * i:2 * i + 2, :],
                                start=(i == 0), stop=(i == KK // 2 - 1),
                                perf_mode=DR)
                        if (c * 3 + p) % 2 == 0:
                            nc.scalar.mul(
                                qk_t[:, c, p * 512:(p + 1) * 512], mmps, C_QK)
                        else:
                            nc.vector.tensor_scalar_mul(
                                qk_t[:, c, p * 512:(p + 1) * 512], mmps, C_QK)

                # covariance + Gram over a pair of tiles: per head
                # [Gq | Gk | C] = [q'q | k'k | q'k], DoubleRow chunk pairs.
                # Emitted before vT on odd tiles so the final flush (and the
                # phase-2 chain it gates) overlaps the last tile's vT matmuls.
                state["qk_pair"].append(qk_t)
                if t % 2 == 1:
                    qk_pair = state["qk_pair"]
                    for h in range(H):
                        cg_ps = ps_mm.tile([HD, 288], FP32, name="cg_ps",
                                           tag="s")
                        np_ = 2 * len(qk_pair)
                        for i in range(np_):
                            qkx = qk_pair[i // 2]
                            lo = (i % 2) * 2
                            q_sl = qkx[:, lo:lo + 2, HD * h:HD * h + HD]
                            k_sl = qkx[:, lo:lo + 2,
                                       C + HD * h:C + HD * h + HD]
                            nc.tensor.matmul(
                                cg_ps[:, 0:HD], q_sl, q_sl,
                                start=(i == 0), stop=False, perf_mode=DR)
                            nc.tensor.matmul(
                                cg_ps[:, HD:2 * HD], k_sl, k_sl,
                                start=False, stop=False, perf_mode=DR)
                            nc.tensor.matmul(
                                cg_ps[:, 2 * HD:3 * HD], q_sl, k_sl,
                                start=False, stop=(i == np_ - 1),
                                perf_mode=DR)
                        nc.vector.tensor_add(
                            cg_accum[:, h, :], cg_ps, cg_accum[:, h, :])
                    state["qk_pair"] = []

            def vtstage(t, xT_bf):
                # vT = w_vT.T @ xT_bf (feature-major, c-major blocks);
                # batched over pairs of tiles to halve strip-DMA count
                if t % 2 == 0:
                    state["vt_sb"] = vtsb.tile([128, KK, 2 * TOK_TILE], BF16,
                                               name="vt_sb")
                vt_sb = state["vt_sb"]
                half = (t % 2) * TOK_TILE
                for m in range(KK):
                    vps = ps_mm.tile([128, TOK_TILE], FP32, name="vps",
                                     tag="s")
                    for kk in range(KK):
                        nc.tensor.matmul(
                            vps, w_vT[:, kk, m * 128:(m + 1) * 128],
                            xT_bf[:, kk, :],
                            start=(kk == 0), stop=(kk == KK - 1))
                    if m % 2 == 0:
                        nc.scalar.copy(
                            vt_sb[:, m, half:half + TOK_TILE], vps)
                    else:
                        nc.vector.tensor_copy(
                            vt_sb[:, m, half:half + TOK_TILE], vps)
                if t % 2 == 1:
                    tp0 = (t - 1) * TOK_TILE
                    for si, (m, p0, run, h, d0) in enumerate(vstrips):
                        src = vt_sb[p0:p0 + run, m, :]
                        dst = vt_all[d0:d0 + run, h, tp0:tp0 + 2 * TOK_TILE]
                        if si % 3 == 0:
                            nc.sync.dma_start(dst, src)
                        elif si % 3 == 1:
                            nc.scalar.dma_start(dst, src)
                        else:
                            nc.gpsimd.dma_start(dst, src)

            # w prep in groups of 4 row-blocks: one wide psum + one cast per
            # (group, kk) instead of 4 narrow ones, using the ps_mm banks
            # that sit idle until the first qk matmul. Interleaved with the
            # first two x stages: the first qk matmul only needs w_qk
            # p-block 0 (m 0..3) + xT(0).
            xT01 = [None, None]
            with tc.tile_pool(name="wload", bufs=2) as wload:
                def wprep(grp):
                    w_blk = wload.tile([128, 2, C], FP32R, name="w_blk")
                    nc.scalar.dma_start(
                        w_blk,
                        wqkv_d[grp * 256:(grp + 1) * 256, :].rearrange(
                            "(b p) f -> p b f", p=128))
                    for kk in range(KK):
                        tps = ps_mm.tile([128, 256], FP32R, name="wps",
                                         tag="s")
                        for b in range(2):
                            nc.tensor.transpose(
                                tps[:, b * 128:(b + 1) * 128],
                                w_blk[:, b, kk * 128:(kk + 1) * 128], ident)
                        if grp < 6:
                            dst = w_qk_f8[grp // 2][
                                :, kk, (grp % 2) * 256:(grp % 2) * 256 + 256]
                            if (grp + kk) % 2 == 0:
                                nc.vector.tensor_scalar_mul(dst, tps, S_W)
                            else:
                                nc.scalar.mul(dst, tps, S_W)
                        else:
                            base = (grp - 6) * 256
                            dst = w_vT[:, kk, base:base + 256]
                            if kk % 2 == 0:
                                nc.vector.tensor_copy(dst, tps)
                            else:
                                nc.scalar.copy(dst, tps)

                wprep(0)
                wprep(1)
                xT01[0] = xstage(0)
                wprep(2)
                wprep(3)
                xT01[1] = xstage(1)
                for grp in range(4, 9):
                    wprep(grp)

            for t in range(N_TILES):
                xT_bf, xT_f8 = xT01[t] if t < 2 else state.pop(("x", t))
                mmstage(t, xT_bf, xT_f8)
                if t < 6:
                    vtstage(t, xT_bf)
                else:
                    state[("xbf", t)] = xT_bf
                if t + 2 < N_TILES:
                    state[("x", t + 2)] = xstage(t + 2)

            phase2(nc, tc, singles, dram, ps_tr, cg_accum, attnT, temp_all,
                   ident96f)

            # deferred vT for the last tile pair: fills the PE while the
            # phase-2 DVE/ACT softmax chain runs
            vtstage(6, state.pop(("xbf", 6)))
            vtstage(7, state.pop(("xbf", 7)))

        # ---------------- phase 3 pools; w_projT prep emitted first so the
        # PE has work while the DVE/ACT-heavy phase 2 chain runs ----------
        with tc.tile_pool(name="wpp", bufs=1) as wpp, \
             tc.tile_pool(name="wpload", bufs=2) as wpload, \
             tc.tile_pool(name="ot96p", bufs=2) as ot96p, \
             tc.tile_pool(name="otp", bufs=2) as otp, \
             tc.tile_pool(name="yp", bufs=2) as yp, \
             tc.tile_pool(name="ps_o", bufs=3, space="PSUM") as ps_o, \
             tc.tile_pool(name="ps_y", bufs=3, space="PSUM") as ps_y:
            # w_proj (cout, c) -> w_projT128 [128, m, cout] (dense c-major)
            w_projT = wpp.tile([128, KK, C], BF16)
            for n in range(KK):
                wp_blk = wpload.tile([128, C], FP32R, name="wp_blk")
                nc.sync.dma_start(wp_blk, wproj_d[n * 128:(n + 1) * 128, :])
                for m in range(KK):
                    tps2 = ps_tr.tile([128, 128], FP32R, name="tps2", tag="tr")
                    nc.tensor.transpose(
                        tps2, wp_blk[:, m * 128:(m + 1) * 128], ident)
                    if (n + m) % 2 == 0:
                        nc.vector.tensor_copy(
                            w_projT[:, m, n * 128:(n + 1) * 128], tps2)
                    else:
                        nc.scalar.copy(
                            w_projT[:, m, n * 128:(n + 1) * 128], tps2)

            # ---------------- phase 3: attn@v + proj, sw-pipelined over
            # 2-tile groups (halves the relayout strip-DMA count) ----------
            ostrips = _ot_strips()
            T2 = 2 * TOK_TILE

            def attnv_group(g):
                g0 = g * T2
                ot96 = ot96p.tile([HD, H, T2], BF16, name="ot96")
                otsb = otp.tile([128, KK, T2], BF16, name="otsb")
                for half in range(2):
                    t0 = g0 + half * TOK_TILE
                    for h in range(H):
                        ops_ = ps_o.tile([HD, TOK_TILE], FP32, name="ops_")
                        nc.tensor.matmul(ops_, attnT[:, h, :],
                                         vt_all[:, h, t0:t0 + TOK_TILE],
                                         start=True, stop=True)
                        hh0 = half * TOK_TILE
                        if h % 2 == 0:
                            nc.vector.tensor_copy(
                                ot96[:, h, hh0:hh0 + TOK_TILE], ops_)
                        else:
                            nc.scalar.copy(
                                ot96[:, h, hh0:hh0 + TOK_TILE], ops_)
                for si, (h, d0, run, m, p0) in enumerate(ostrips):
                    src = ot96[d0:d0 + run, h, :]
                    dst = otsb[p0:p0 + run, m, :]
                    if si % 3 == 0:
                        nc.sync.dma_start(dst, src)
                    elif si % 3 == 1:
                        nc.scalar.dma_start(dst, src)
                    else:
                        nc.gpsimd.dma_start(dst, src)
                return otsb

            def proj_group(g, otsb):
                for piece in range(CHUNKS):
                    t0 = g * T2 + piece * 256
                    y_t = yp.tile([128, 2, C], FP32, name="y_t")
                    for c in range(2):
                        cc = piece * 2 + c
                        for (off, width) in ((0, 512), (512, 256)):
                            yps = ps_y.tile([128, 512], FP32, name="yps")
                            for m in range(KK):
                                nc.tensor.matmul(
                                    yps[:, :width],
                                    otsb[:, m, cc * 128:(cc + 1) * 128],
                                    w_projT[:, m, off:off + width],
                                    start=(m == 0), stop=(m == KK - 1))
                            nc.vector.tensor_tensor(
                                y_t[:, c, off:off + width], yps[:, :width],
                                b_all[:, off:off + width], mybir.AluOpType.add)
                    nc.scalar.dma_start(
                        out_d[t0:t0 + 256, :].rearrange(
                            "(c p) f -> p c f", p=128),
                        y_t)

            pend = None
            for g in range(N_TILES // 2):
                cur = attnv_group(g)
                if pend is not None:
                    proj_group(*pend)
                pend = (g, cur)
            proj_group(*pend)


def _get_nc():
    global _CACHED_NC
    if _CACHED_NC is None:
        _CACHED_NC = build_nc()
    return _CACHED_NC


def kernel(x, w_qkv, temperature, w_proj, b_proj):
    nc = _get_nc()
    x = np.ascontiguousarray(np.asarray(x, dtype=np.float32))
    in_maps = []
    for b in range(8):
        in_maps.append({
            "x": x[b],
            "w_qkv": np.asarray(w_qkv, dtype=np.float32),
            "temperature": np.asarray(temperature, dtype=np.float32),
            "w_proj": np.asarray(w_proj, dtype=np.float32),
            "b_proj": np.asarray(b_proj, dtype=np.float32),
        })
    res = run_bass_kernel_spmd(nc, in_maps, core_ids=list(range(8)))
    return np.stack([r["out"] for r in res.results], axis=0)


# revision 33
# speedup vs baseline: 1.7705x; 1.0412x over previous
"""CrossCovarianceAttn Trainium2 kernel.

Data-parallel over B=8 across 8 NeuronCores; each core runs the full model on
one batch element.

Numerics: the q/k projection and the per-head covariance/Gram matmuls run in
fp8e4m3 with perf_mode=DoubleRow (two 128-row k-tiles per instruction, 0.5
cyc/row) — safe because q,k are l2-normalized over the token dim downstream,
which cancels the fp8 scaling exactly and the softmax logits are small
(|logit| <= temperature by Cauchy-Schwarz, which also lets softmax skip the
max-subtraction). The v projection and attn@v run in bf16; the output
projection runs fp32r on a dense 128-row contraction. PE transposes run fp32r
(1.5 cyc/row; tf32-ish truncation is harmless, everything transposed feeds
bf16/fp8 or is attn in [0,1]).

Cross-partition relayouts use DMA (compute engines cannot cross partitions):
  - vT (c-major [128,6,N] bf16 from the PE) -> vt_all [96,H,N] per-head
    layout via sbuf->sbuf strip DMAs.
  - attn@v psums ([96,512] per head) -> otsb128 [128,6,512] fp32r via
    gpsimd psum->sbuf strip DMAs (fp32->fp32r is a bit-identical
    reinterpret, the only cast SWDGE can do), so the projection contracts
    dense 128-row blocks instead of 96-row head blocks.

Pipeline per core:
  phase 0: w_qkv -> w_qk_f8 (x64, plain layout, 3 per-512-col tiles so the
           first qk matmul only waits on a third of the prep) + w_vT bf16
  phase 1: per 512-token tile: PE-transpose x -> xT (fp32r); cast to xT_bf
           (DVE/ACT) and xT_bf -> xT_f8 on GpSimd (x32); qk = xT_f8.T @
           w_qk_f8 (DoubleRow) -> qk_t fp8 (x 26/1135); vT = w_vT.T @ xT_bf
           -> strip-DMA into vt_all; covariance+Gram (3 DoubleRow matmuls
           per head: Gq | C | Gk) accumulated over pairs of tiles, flushed
           into cg_accum fp32
  phase 2: Gram diagonals -> 1/max(||.||,eps); batched all-head softmax
           (no max-sub); transpose attn -> attnT bf16
  phase 3 (sw-pipelined): attn_h @ vt_all_h -> otsb128 strips;
           y = otsb128.T @ w_projT128 + b -> out
"""
import os
import sys

sys.path.insert(0, "/opt/trn_rl_repo")

import numpy as np

import concourse.bass as bass
import concourse.mybir as mybir
import concourse.tile as tile
from concourse import bacc
from concourse.bass_utils import run_bass_kernel_spmd
from concourse.masks import make_identity

FP32 = mybir.dt.float32
FP32R = mybir.dt.float32r
BF16 = mybir.dt.bfloat16
FP8 = mybir.dt.float8e4
DR = mybir.MatmulPerfMode.DoubleRow

N_TOK = 4096
C = 768
H = 8
HD = 96
C3 = 3 * C
TOK_TILE = 512
N_TILES = N_TOK // TOK_TILE
CHUNKS = TOK_TILE // 128
KK = C // 128
EPS = 1e-12

S_X = 32.0            # x -> fp8 scale
S_W = 64.0            # w_q/w_k -> fp8 scale
C_QK = 26.0 / 1135.0  # qk psum (scaled x2048) -> fp8 scale

_CACHED_NC = None
XTP_BUFS = int(os.environ.get("XTP_BUFS", "2"))
QKP_BUFS = int(os.environ.get("QKP_BUFS", "3"))
PSTR_BUFS = int(os.environ.get("PSTR_BUFS", "2"))
PSMM_BUFS = int(os.environ.get("PSMM_BUFS", "6"))
XIN_BUFS = int(os.environ.get("XIN_BUFS", "2"))


def _vt_strips():
    """Strips (m, p0, run, h, d0): vt_sb partition p of block m holds
    v-feature c = 128m + p = 96h + d -> vt_all partition d, head h."""
    strips = []
    for m in range(KK):
        c0 = 128 * m
        p = 0
        while p < 128:
            h, d = divmod(c0 + p, HD)
            run = min(128 - p, HD - d)
            strips.append((m, p, run, h, d))
            p += run
    return strips


def _ot_strips():
    """Strips (h, d0, run, m, p0): attnv psum for head h, row d holds
    out-feature c = 96h + d -> otsb128 partition p = c % 128, block m."""
    strips = []
    for h in range(H):
        c0 = HD * h
        d = 0
        while d < HD:
            m, p = divmod(c0 + d, 128)
            run = min(HD - d, 128 - p)
            strips.append((h, d, run, m, p))
            d += run
    return strips


def phase2(nc, tc, singles, dram, ps_tr, cg_accum, attnT, temp_all, ident96f):
    """Norms + batched all-head softmax -> attnT (bf16).

    cg_accum[:, h, :]: [0:96] Gq, [96:192] Gk, [192:288] C. The Gq|Gk
    adjacency lets one mask-mult + one reduce grab both sets of diagonals.
    |logit| <= temperature (Cauchy-Schwarz on unit vectors), so exp runs
    directly with no max-subtraction.
    """
    import concourse.mybir as mybir

    identb = ident96f[:, None, None, :].to_broadcast((HD, H, 2, HD))
    sq = singles.tile([HD, H, 2], FP32)
    scr = singles.tile([HD, H, 2, HD], FP32)
    nc.vector.tensor_tensor(
        scr, cg_accum[:, :, 0:2 * HD].rearrange(
            "d h (two e) -> d h two e", two=2),
        identb, mybir.AluOpType.mult)
    nc.vector.reduce_sum(sq[:, :, :, None], scr, axis=mybir.AxisListType.X)

    nrm = singles.tile([HD, H, 2], FP32)
    nc.scalar.sqrt(nrm, sq)
    nc.vector.tensor_scalar_max(nrm, nrm, EPS)
    rnorm = singles.tile([HD, H, 2], FP32)
    nc.vector.reciprocal(rnorm, nrm)
    rq = singles.tile([HD, H], FP32)
    nc.vector.tensor_tensor(rq, rnorm[:, :, 0], temp_all,
                            mybir.AluOpType.mult)

    # rk to the free dim: store h-major to DRAM, broadcast-read back
    rk_scr = dram.tile([H, HD], FP32)
    nc.sync.dma_start(
        bass.AP(tensor=rk_scr.tensor, offset=rk_scr.offset,
                ap=[[1, HD], [HD, H]]),
        rnorm[:, :, 1])
    rk_all = singles.tile([HD, H, HD], FP32)
    nc.sync.dma_start(
        rk_all, bass.AP(tensor=rk_scr.tensor, offset=rk_scr.offset,
                        ap=[[0, HD], [1, H * HD]]))

    attL = singles.tile([HD, H, HD], FP32)
    nc.vector.tensor_tensor(
        attL, cg_accum[:, :, 2 * HD:3 * HD],
        rq[:, :, None].to_broadcast((HD, H, HD)), mybir.AluOpType.mult)
    nc.vector.tensor_tensor(attL, attL, rk_all, mybir.AluOpType.mult)
    nc.scalar.activation(attL, attL, mybir.ActivationFunctionType.Exp)
    sea = singles.tile([HD, H, 1], FP32)
    nc.vector.reduce_sum(sea, attL, axis=mybir.AxisListType.X)
    rsea = singles.tile([HD, H, 1], FP32)
    nc.vector.reciprocal(rsea, sea)
    nc.vector.tensor_tensor(
        attL, attL, rsea.to_broadcast((HD, H, HD)), mybir.AluOpType.mult)
    for h in range(H):
        atps = ps_tr.tile([HD, HD], FP32, name="atps", tag="tr")
        nc.tensor.transpose(atps, attL[:, h, :], ident96f)
        if h % 2 == 0:
            nc.vector.tensor_copy(attnT[:, h, :], atps)
        else:
            nc.scalar.copy(attnT[:, h, :], atps)


def build_nc():
    nc = bacc.Bacc("TRN2", target_bir_lowering=False, debug=False, num_devices=8)

    x_d = nc.dram_tensor("x", (N_TOK, C), FP32R, kind="ExternalInput").ap()
    wqkv_d = nc.dram_tensor("w_qkv", (C3, C), FP32R, kind="ExternalInput").ap()
    temp_d = nc.dram_tensor("temperature", (H, 1, 1), FP32, kind="ExternalInput").ap()
    wproj_d = nc.dram_tensor("w_proj", (C, C), FP32R, kind="ExternalInput").ap()
    bproj_d = nc.dram_tensor("b_proj", (C,), FP32, kind="ExternalInput").ap()
    out_d = nc.dram_tensor("out", (N_TOK, C), FP32, kind="ExternalOutput").ap()

    with tile.TileContext(nc) as tc:
        _build(tc, nc, x_d, wqkv_d, temp_d, wproj_d, bproj_d, out_d)
    nc.compile()
    return nc


def _build(tc, nc, x_d, wqkv_d, temp_d, wproj_d, bproj_d, out_d):
    import contextlib

    ctx = contextlib.ExitStack()
    with ctx:
        singles = ctx.enter_context(tc.tile_pool(name="singles", bufs=1))
        dram = ctx.enter_context(tc.tile_pool(name="dram", bufs=1, space="DRAM"))
        ps_tr = ctx.enter_context(tc.tile_pool(name="ps_tr", bufs=PSTR_BUFS, space="PSUM"))

        ident_f32 = singles.tile([128, 128], FP32)
        make_identity(nc, ident_f32)
        ident = singles.tile([128, 128], BF16)
        nc.vector.tensor_copy(ident, ident_f32)
        ident96f = ident_f32[0:HD, 0:HD]

        b_all = singles.tile([128, C], FP32)
        nc.gpsimd.dma_start(
            b_all, bass.AP(tensor=bproj_d.tensor, offset=bproj_d.offset,
                           ap=[[0, 128], [1, C]]))
        temp_all = singles.tile([HD, H], FP32)
        nc.gpsimd.dma_start(
            temp_all, bass.AP(tensor=temp_d.tensor, offset=temp_d.offset,
                              ap=[[0, HD], [1, H]]))

        warm = singles.tile([1, 1], FP32)
        nc.vector.memset(warm, 0.5)
        nc.scalar.activation(warm, warm, mybir.ActivationFunctionType.Exp)
        nc.scalar.sqrt(warm, warm)

        cg_accum = singles.tile([HD, H, 288], FP32)
        nc.vector.memset(cg_accum, 0.0)
        attnT = singles.tile([HD, H, HD], BF16)
        vt_all = singles.tile([HD, H, N_TOK], BF16)

        # ---------------- phase 0 + 1 ----------------
        # w_qk plain layout: w_qk_f8[p][:, kk, j] = 64 * w_qkv[512p+j, 128kk+:]
        # (cols 0..767 across the 3 tiles = q rows, 768..1535 = k rows)
        with tc.tile_pool(name="wqk_pool", bufs=1) as wqk_pool, \
             tc.tile_pool(name="xin", bufs=XIN_BUFS) as xin, \
             tc.tile_pool(name="xtp", bufs=XTP_BUFS) as xtp, \
             tc.tile_pool(name="qkp", bufs=QKP_BUFS) as qkp, \
             tc.tile_pool(name="vtsb", bufs=2) as vtsb, \
             tc.tile_pool(name="ps_mm", bufs=PSMM_BUFS, space="PSUM") as ps_mm:
            w_qk_f8 = [wqk_pool.tile([128, KK, 512], FP8, name=f"wqk{p}")
                       for p in range(3)]
            w_vT = wqk_pool.tile([128, KK, C], BF16)
            vstrips = _vt_strips()
            state = {"vt_sb": None, "qk_pair": []}

            def xstage(t):
                """x load + bf16 pre-cast + PE transpose + bf16/fp8 casts."""
                t0 = t * TOK_TILE
                x_t = xin.tile([128, CHUNKS, C], BF16, name="x_t")
                nc.gpsimd.dma_start(
                    x_t, x_d[t0:t0 + TOK_TILE, :].rearrange(
                        "(c p) f -> p c f", p=128))
                xT_bf = xtp.tile([128, KK, TOK_TILE], BF16, name="xT_bf")
                xT_f8 = xtp.tile([128, KK, TOK_TILE], FP8, name="xT_f8")
                for kk in range(KK):
                    xps = ps_tr.tile([128, TOK_TILE], BF16, name="xps",
                                     tag="tr")
                    for c in range(CHUNKS):
                        nc.tensor.transpose(
                            xps[:, c * 128:(c + 1) * 128],
                            x_t[:, c, kk * 128:(kk + 1) * 128], ident)
                    if kk % 2 == 0:
                        nc.vector.tensor_copy(xT_bf[:, kk, :], xps)
                        nc.scalar.mul(xT_f8[:, kk, :], xps, S_X)
                    else:
                        nc.scalar.copy(xT_bf[:, kk, :], xps)
                        nc.vector.tensor_scalar_mul(
                            xT_f8[:, kk, :], xps, S_X)
                return xT_bf, xT_f8

            def mmstage(t, xT_bf, xT_f8):
                """qk + vT matmuls, vt strips, covariance for one tile."""
                t0 = t * TOK_TILE
                qk_t = qkp.tile([128, CHUNKS, 1536], FP8, name="qk_t")

                # qk = xT.T @ w_qk (token-major; fp8 DoubleRow pairs)
                for c in range(CHUNKS):
                    for p in range(3):
                        mmps = ps_mm.tile([128, 512], FP32, name="mmps",
                                          tag="s")
                        for i in range(KK // 2):
                            nc.tensor.matmul(
                                mmps,
                                xT_f8[:, 2 * i:2 * i + 2,
                                      c * 128:(c + 1) * 128],
                                w_qk_f8[p][:, 2 * i:2 * i + 2, :],
                                start=(i == 0), stop=(i == KK // 2 - 1),
                                perf_mode=DR)
                        if (c * 3 + p) % 2 == 0:
                            nc.scalar.mul(
                                qk_t[:, c, p * 512:(p + 1) * 512], mmps, C_QK)
                        else:
                            nc.vector.tensor_scalar_mul(
                                qk_t[:, c, p * 512:(p + 1) * 512], mmps, C_QK)

                # covariance + Gram over a pair of tiles: per head
                # [Gq | Gk | C] = [q'q | k'k | q'k], DoubleRow chunk pairs.
                # Emitted before vT on odd tiles so the final flush (and the
                # phase-2 chain it gates) overlaps the last tile's vT matmuls.
                state["qk_pair"].append(qk_t)
                if t % 2 == 1:
                    qk_pair = state["qk_pair"]
                    for h in range(H):
                        cg_ps = ps_mm.tile([HD, 288], FP32, name="cg_ps",
                                           tag="s")
                        np_ = 2 * len(qk_pair)
                        for i in range(np_):
                            qkx = qk_pair[i // 2]
                            lo = (i % 2) * 2
                            q_sl = qkx[:, lo:lo + 2, HD * h:HD * h + HD]
                            k_sl = qkx[:, lo:lo + 2,
                                       C + HD * h:C + HD * h + HD]
                            nc.tensor.matmul(
                                cg_ps[:, 0:HD], q_sl, q_sl,
                                start=(i == 0), stop=False, perf_mode=DR)
                            nc.tensor.matmul(
                                cg_ps[:, HD:2 * HD], k_sl, k_sl,
                                start=False, stop=False, perf_mode=DR)
                            nc.tensor.matmul(
                                cg_ps[:, 2 * HD:3 * HD], q_sl, k_sl,
                                start=False, stop=(i == np_ - 1),
                                perf_mode=DR)
                        nc.vector.tensor_add(
                            cg_accum[:, h, :], cg_ps, cg_accum[:, h, :])
                    state["qk_pair"] = []

            def vtstage(t, xT_bf):
                # vT = w_vT.T @ xT_bf (feature-major, c-major blocks);
                # batched over pairs of tiles to halve strip-DMA count
                if t % 2 == 0:
                    state["vt_sb"] = vtsb.tile([128, KK, 2 * TOK_TILE], BF16,
                                               name="vt_sb")
                vt_sb = state["vt_sb"]
                half = (t % 2) * TOK_TILE
                for m in range(KK):
                    vps = ps_mm.tile([128, TOK_TILE], FP32, name="vps",
                                     tag="s")
                    for kk in range(KK):
                        nc.tensor.matmul(
                            vps, w_vT[:, kk, m * 128:(m + 1) * 128],
                            xT_bf[:, kk, :],
                            start=(kk == 0), stop=(kk == KK - 1))
                    if m % 2 == 0:
                        nc.scalar.copy(
                            vt_sb[:, m, half:half + TOK_TILE], vps)
                    else:
                        nc.vector.tensor_copy(
                            vt_sb[:, m, half:half + TOK_TILE], vps)
                if t % 2 == 1:
                    tp0 = (t - 1) * TOK_TILE
                    for si, (m, p0, run, h, d0) in enumerate(vstrips):
                        src = vt_sb[p0:p0 + run, m, :]
                        dst = vt_all[d0:d0 + run, h, tp0:tp0 + 2 * TOK_TILE]
                        if si % 3 == 0:
                            nc.sync.dma_start(dst, src)
                        elif si % 3 == 1:
                            nc.scalar.dma_start(dst, src)
                        else:
                            nc.gpsimd.dma_start(dst, src)

            # w prep in groups of 4 row-blocks: one wide psum + one cast per
            # (group, kk) instead of 4 narrow ones, using the ps_mm banks
            # that sit idle until the first qk matmul. Interleaved with the
            # first two x stages: the first qk matmul only needs w_qk
            # p-block 0 (m 0..3) + xT(0).
            xT01 = [None, None]
            with tc.tile_pool(name="wload", bufs=2) as wload:
                def wprep(grp):
                    w_blk = wload.tile([128, 2, C], BF16, name="w_blk")
                    nc.gpsimd.dma_start(
                        w_blk,
                        wqkv_d[grp * 256:(grp + 1) * 256, :].rearrange(
                            "(b p) f -> p b f", p=128))
                    for kk in range(KK):
                        tps = ps_mm.tile([128, 256], BF16, name="wps",
                                         tag="s")
                        for b in range(2):
                            nc.tensor.transpose(
                                tps[:, b * 128:(b + 1) * 128],
                                w_blk[:, b, kk * 128:(kk + 1) * 128], ident)
                        if grp < 6:
                            dst = w_qk_f8[grp // 2][
                                :, kk, (grp % 2) * 256:(grp % 2) * 256 + 256]
                            if (grp + kk) % 2 == 0:
                                nc.vector.tensor_scalar_mul(dst, tps, S_W)
                            else:
                                nc.scalar.mul(dst, tps, S_W)
                        else:
                            base = (grp - 6) * 256
                            dst = w_vT[:, kk, base:base + 256]
                            if kk % 2 == 0:
                                nc.vector.tensor_copy(dst, tps)
                            else:
                                nc.scalar.copy(dst, tps)

                xT01[0] = xstage(0)
                wprep(0)
                wprep(1)
                xT01[1] = xstage(1)
                wprep(2)
                wprep(3)
                for grp in range(4, 9):
                    wprep(grp)

            for t in range(N_TILES):
                xT_bf, xT_f8 = xT01[t] if t < 2 else state.pop(("x", t))
                mmstage(t, xT_bf, xT_f8)
                if t < 6:
                    vtstage(t, xT_bf)
                else:
                    state[("xbf", t)] = xT_bf
                if t + 2 < N_TILES:
                    state[("x", t + 2)] = xstage(t + 2)

            phase2(nc, tc, singles, dram, ps_tr, cg_accum, attnT, temp_all,
                   ident96f)

            # deferred vT for the last tile pair: fills the PE while the
            # phase-2 DVE/ACT softmax chain runs
            vtstage(6, state.pop(("xbf", 6)))
            vtstage(7, state.pop(("xbf", 7)))

        # ---------------- phase 3 pools; w_projT prep emitted first so the
        # PE has work while the DVE/ACT-heavy phase 2 chain runs ----------
        with tc.tile_pool(name="wpp", bufs=1) as wpp, \
             tc.tile_pool(name="wpload", bufs=2) as wpload, \
             tc.tile_pool(name="ot96p", bufs=2) as ot96p, \
             tc.tile_pool(name="otp", bufs=2) as otp, \
             tc.tile_pool(name="yp", bufs=2) as yp, \
             tc.tile_pool(name="ps_o", bufs=3, space="PSUM") as ps_o, \
             tc.tile_pool(name="ps_y", bufs=3, space="PSUM") as ps_y:
            # w_proj (cout, c) -> w_projT128 [128, m, cout] (dense c-major)
            w_projT = wpp.tile([128, KK, C], BF16)
            for n in range(KK):
                wp_blk = wpload.tile([128, C], BF16, name="wp_blk")
                nc.gpsimd.dma_start(wp_blk, wproj_d[n * 128:(n + 1) * 128, :])
                for m in range(KK):
                    tps2 = ps_tr.tile([128, 128], BF16, name="tps2", tag="tr")
                    nc.tensor.transpose(
                        tps2, wp_blk[:, m * 128:(m + 1) * 128], ident)
                    if (n + m) % 2 == 0:
                        nc.vector.tensor_copy(
                            w_projT[:, m, n * 128:(n + 1) * 128], tps2)
                    else:
                        nc.scalar.copy(
                            w_projT[:, m, n * 128:(n + 1) * 128], tps2)

            # ---------------- phase 3: attn@v + proj, sw-pipelined over
            # 2-tile groups (halves the relayout strip-DMA count) ----------
            ostrips = _ot_strips()
            T2 = 2 * TOK_TILE

            def attnv_group(g):
                g0 = g * T2
                ot96 = ot96p.tile([HD, H, T2], BF16, name="ot96")
                otsb = otp.tile([128, KK, T2], BF16, name="otsb")
                for half in range(2):
                    t0 = g0 + half * TOK_TILE
                    for h in range(H):
                        ops_ = ps_o.tile([HD, TOK_TILE], FP32, name="ops_")
                        nc.tensor.matmul(ops_, attnT[:, h, :],
                                         vt_all[:, h, t0:t0 + TOK_TILE],
                                         start=True, stop=True)
                        hh0 = half * TOK_TILE
                        if h % 2 == 0:
                            nc.vector.tensor_copy(
                                ot96[:, h, hh0:hh0 + TOK_TILE], ops_)
                        else:
                            nc.scalar.copy(
                                ot96[:, h, hh0:hh0 + TOK_TILE], ops_)
                for si, (h, d0, run, m, p0) in enumerate(ostrips):
                    src = ot96[d0:d0 + run, h, :]
                    dst = otsb[p0:p0 + run, m, :]
                    if si % 3 == 0:
                        nc.sync.dma_start(dst, src)
                    elif si % 3 == 1:
                        nc.scalar.dma_start(dst, src)
                    else:
                        nc.gpsimd.dma_start(dst, src)
                return otsb

            def proj_group(g, otsb):
                for piece in range(CHUNKS):
                    t0 = g * T2 + piece * 256
                    y_t = yp.tile([128, 2, C], FP32, name="y_t")
                    for c in range(2):
                        cc = piece * 2 + c
                        for (off, width) in ((0, 512), (512, 256)):
                            yps = ps_y.tile([128, 512], FP32, name="yps")
                            for m in range(KK):
                                nc.tensor.matmul(
                                    yps[:, :width],
                                    otsb[:, m, cc * 128:(cc + 1) * 128],
                                    w_projT[:, m, off:off + width],
                                    start=(m == 0), stop=(m == KK - 1))
                            nc.vector.tensor_tensor(
                                y_t[:, c, off:off + width], yps[:, :width],
                                b_all[:, off:off + width], mybir.AluOpType.add)
                    nc.scalar.dma_start(
                        out_d[t0:t0 + 256, :].rearrange(
                            "(c p) f -> p c f", p=128),
                        y_t)

            pend = None
            for g in range(N_TILES // 2):
                cur = attnv_group(g)
                if pend is not None:
                    proj_group(*pend)
                pend = (g, cur)
            proj_group(*pend)


def _get_nc():
    global _CACHED_NC
    if _CACHED_NC is None:
        _CACHED_NC = build_nc()
    return _CACHED_NC


def kernel(x, w_qkv, temperature, w_proj, b_proj):
    nc = _get_nc()
    x = np.ascontiguousarray(np.asarray(x, dtype=np.float32))
    in_maps = []
    for b in range(8):
        in_maps.append({
            "x": x[b],
            "w_qkv": np.asarray(w_qkv, dtype=np.float32),
            "temperature": np.asarray(temperature, dtype=np.float32),
            "w_proj": np.asarray(w_proj, dtype=np.float32),
            "b_proj": np.asarray(b_proj, dtype=np.float32),
        })
    res = run_bass_kernel_spmd(nc, in_maps, core_ids=list(range(8)))
    return np.stack([r["out"] for r in res.results], axis=0)


# revision 44
# speedup vs baseline: 1.8166x; 1.0260x over previous
"""CrossCovarianceAttn Trainium2 kernel.

Data-parallel over B=8 across 8 NeuronCores; each core runs the full model on
one batch element.

Numerics: the q/k projection and the per-head covariance/Gram matmuls run in
fp8e4m3 with perf_mode=DoubleRow (two 128-row k-tiles per instruction, 0.5
cyc/row) — safe because q,k are l2-normalized over the token dim downstream,
which cancels the fp8 scaling exactly and the softmax logits are small
(|logit| <= temperature by Cauchy-Schwarz, which also lets softmax skip the
max-subtraction). The v projection and attn@v run in bf16; the output
projection runs fp32r on a dense 128-row contraction. PE transposes run fp32r
(1.5 cyc/row; tf32-ish truncation is harmless, everything transposed feeds
bf16/fp8 or is attn in [0,1]).

Cross-partition relayouts use DMA (compute engines cannot cross partitions):
  - vT (c-major [128,6,N] bf16 from the PE) -> vt_all [96,H,N] per-head
    layout via sbuf->sbuf strip DMAs.
  - attn@v psums ([96,512] per head) -> otsb128 [128,6,512] fp32r via
    gpsimd psum->sbuf strip DMAs (fp32->fp32r is a bit-identical
    reinterpret, the only cast SWDGE can do), so the projection contracts
    dense 128-row blocks instead of 96-row head blocks.

Pipeline per core:
  phase 0: w_qkv -> w_qk_f8 (x64, plain layout, 3 per-512-col tiles so the
           first qk matmul only waits on a third of the prep) + w_vT bf16
  phase 1: per 512-token tile: PE-transpose x -> xT (fp32r); cast to xT_bf
           (DVE/ACT) and xT_bf -> xT_f8 on GpSimd (x32); qk = xT_f8.T @
           w_qk_f8 (DoubleRow) -> qk_t fp8 (x 26/1135); vT = w_vT.T @ xT_bf
           -> strip-DMA into vt_all; covariance+Gram (3 DoubleRow matmuls
           per head: Gq | C | Gk) accumulated over pairs of tiles, flushed
           into cg_accum fp32
  phase 2: Gram diagonals -> 1/max(||.||,eps); batched all-head softmax
           (no max-sub); transpose attn -> attnT bf16
  phase 3 (sw-pipelined): attn_h @ vt_all_h -> otsb128 strips;
           y = otsb128.T @ w_projT128 + b -> out
"""
import os
import sys

sys.path.insert(0, "/opt/trn_rl_repo")

import numpy as np

import concourse.bass as bass
import concourse.mybir as mybir
import concourse.tile as tile
from concourse import bacc
from concourse.bass_utils import run_bass_kernel_spmd
from concourse.masks import make_identity

FP32 = mybir.dt.float32
FP32R = mybir.dt.float32r
BF16 = mybir.dt.bfloat16
FP8 = mybir.dt.float8e4
DR = mybir.MatmulPerfMode.DoubleRow

N_TOK = 4096
C = 768
H = 8
HD = 96
C3 = 3 * C
TOK_TILE = 512
N_TILES = N_TOK // TOK_TILE
CHUNKS = TOK_TILE // 128
KK = C // 128
EPS = 1e-12

S_W = 64.0           # w_q/w_k (and w_v) -> fp8/bf16 scale
C_QK = 26.0 / 35.5   # qk psum (scaled x64) -> fp8 scale

_CACHED_NC = None
XTP_BUFS = int(os.environ.get("XTP_BUFS", "3"))
QKP_BUFS = int(os.environ.get("QKP_BUFS", "3"))
PSTR_BUFS = int(os.environ.get("PSTR_BUFS", "2"))
PSMM_BUFS = int(os.environ.get("PSMM_BUFS", "6"))
XIN_BUFS = int(os.environ.get("XIN_BUFS", "3"))


def _vt_strips():
    """Strips (m, p0, run, h, d0): vt_sb partition p of block m holds
    v-feature c = 128m + p = 96h + d -> vt_all partition d, head h."""
    strips = []
    for m in range(KK):
        c0 = 128 * m
        p = 0
        while p < 128:
            h, d = divmod(c0 + p, HD)
            run = min(128 - p, HD - d)
            strips.append((m, p, run, h, d))
            p += run
    return strips


def _ot_strips():
    """Strips (h, d0, run, m, p0): attnv psum for head h, row d holds
    out-feature c = 96h + d -> otsb128 partition p = c % 128, block m."""
    strips = []
    for h in range(H):
        c0 = HD * h
        d = 0
        while d < HD:
            m, p = divmod(c0 + d, 128)
            run = min(HD - d, 128 - p)
            strips.append((h, d, run, m, p))
            d += run
    return strips


def phase2(nc, tc, singles, dram, ps_tr, cg_accum, attnT, temp_all, ident96f):
    """Norms + batched all-head softmax -> attnT (bf16).

    cg_accum[:, h, :]: [0:96] Gq, [96:192] Gk, [192:288] C. The Gq|Gk
    adjacency lets one mask-mult + one reduce grab both sets of diagonals.
    |logit| <= temperature (Cauchy-Schwarz on unit vectors), so exp runs
    directly with no max-subtraction.
    """
    import concourse.mybir as mybir

    identb = ident96f[:, None, None, :].to_broadcast((HD, H, 2, HD))
    sq = singles.tile([HD, H, 2], FP32)
    scr = singles.tile([HD, H, 2, HD], FP32)
    nc.vector.tensor_tensor(
        scr, cg_accum[:, :, 0:2 * HD].rearrange(
            "d h (two e) -> d h two e", two=2),
        identb, mybir.AluOpType.mult)
    nc.vector.reduce_sum(sq[:, :, :, None], scr, axis=mybir.AxisListType.X)

    nrm = singles.tile([HD, H, 2], FP32)
    nc.scalar.sqrt(nrm, sq)
    nc.vector.tensor_scalar_max(nrm, nrm, EPS)
    rnorm = singles.tile([HD, H, 2], FP32)
    nc.vector.reciprocal(rnorm, nrm)
    rq = singles.tile([HD, H], FP32)
    nc.vector.tensor_tensor(rq, rnorm[:, :, 0], temp_all,
                            mybir.AluOpType.mult)

    # rk to the free dim: store h-major to DRAM, broadcast-read back
    rk_scr = dram.tile([H, HD], FP32)
    nc.sync.dma_start(
        bass.AP(tensor=rk_scr.tensor, offset=rk_scr.offset,
                ap=[[1, HD], [HD, H]]),
        rnorm[:, :, 1])
    rk_all = singles.tile([HD, H, HD], FP32)
    nc.sync.dma_start(
        rk_all, bass.AP(tensor=rk_scr.tensor, offset=rk_scr.offset,
                        ap=[[0, HD], [1, H * HD]]))

    attL = singles.tile([HD, H, HD], FP32)
    nc.vector.tensor_tensor(
        attL, cg_accum[:, :, 2 * HD:3 * HD],
        rq[:, :, None].to_broadcast((HD, H, HD)), mybir.AluOpType.mult)
    nc.vector.tensor_tensor(attL, attL, rk_all, mybir.AluOpType.mult)
    nc.scalar.activation(attL, attL, mybir.ActivationFunctionType.Exp)
    sea = singles.tile([HD, H, 1], FP32)
    nc.vector.reduce_sum(sea, attL, axis=mybir.AxisListType.X)
    rsea = singles.tile([HD, H, 1], FP32)
    nc.vector.reciprocal(rsea, sea)
    nc.vector.tensor_tensor(
        attL, attL, rsea.to_broadcast((HD, H, HD)), mybir.AluOpType.mult)
    for h in range(H):
        atps = ps_tr.tile([HD, HD], FP32, name="atps", tag="tr")
        nc.tensor.transpose(atps, attL[:, h, :], ident96f)
        if h % 2 == 0:
            nc.vector.tensor_copy(attnT[:, h, :], atps)
        else:
            nc.scalar.copy(attnT[:, h, :], atps)


def build_nc():
    nc = bacc.Bacc("TRN2", target_bir_lowering=False, debug=False, num_devices=8)

    x_d = nc.dram_tensor("x", (N_TOK, C), FP32R, kind="ExternalInput").ap()
    wqkv_d = nc.dram_tensor("w_qkv", (C3, C), FP32R, kind="ExternalInput").ap()
    temp_d = nc.dram_tensor("temperature", (H, 1, 1), FP32, kind="ExternalInput").ap()
    wproj_d = nc.dram_tensor("w_proj", (C, C), FP32R, kind="ExternalInput").ap()
    bproj_d = nc.dram_tensor("b_proj", (C,), FP32, kind="ExternalInput").ap()
    out_d = nc.dram_tensor("out", (N_TOK, C), FP32, kind="ExternalOutput").ap()

    with tile.TileContext(nc) as tc:
        _build(tc, nc, x_d, wqkv_d, temp_d, wproj_d, bproj_d, out_d)
    nc.compile()
    return nc


def _build(tc, nc, x_d, wqkv_d, temp_d, wproj_d, bproj_d, out_d):
    import contextlib

    ctx = contextlib.ExitStack()
    with ctx:
        singles = ctx.enter_context(tc.tile_pool(name="singles", bufs=1))
        dram = ctx.enter_context(tc.tile_pool(name="dram", bufs=1, space="DRAM"))
        ps_tr = ctx.enter_context(tc.tile_pool(name="ps_tr", bufs=PSTR_BUFS, space="PSUM"))

        ident_f32 = singles.tile([128, 128], FP32)
        make_identity(nc, ident_f32)
        ident = singles.tile([128, 128], BF16)
        nc.vector.tensor_copy(ident, ident_f32)
        ident96f = ident_f32[0:HD, 0:HD]

        b_all = singles.tile([128, C], FP32)
        nc.gpsimd.dma_start(
            b_all, bass.AP(tensor=bproj_d.tensor, offset=bproj_d.offset,
                           ap=[[0, 128], [1, C]]))
        temp_all = singles.tile([HD, H], FP32)
        nc.gpsimd.dma_start(
            temp_all, bass.AP(tensor=temp_d.tensor, offset=temp_d.offset,
                              ap=[[0, HD], [1, H]]))

        warm = singles.tile([1, 1], FP32)
        nc.vector.memset(warm, 0.5)
        nc.scalar.activation(warm, warm, mybir.ActivationFunctionType.Exp)
        nc.scalar.sqrt(warm, warm)

        cg_accum = singles.tile([HD, H, 288], FP32)
        nc.vector.memset(cg_accum, 0.0)
        attnT = singles.tile([HD, H, HD], BF16)
        vt_all = singles.tile([HD, H, N_TOK], BF16)

        # ---------------- phase 0 + 1 ----------------
        # w_qk plain layout: w_qk_f8[p][:, kk, j] = 64 * w_qkv[512p+j, 128kk+:]
        # (cols 0..767 across the 3 tiles = q rows, 768..1535 = k rows)
        with tc.tile_pool(name="wqk_pool", bufs=1) as wqk_pool, \
             tc.tile_pool(name="xin", bufs=XIN_BUFS) as xin, \
             tc.tile_pool(name="xtp", bufs=XTP_BUFS) as xtp, \
             tc.tile_pool(name="qkp", bufs=QKP_BUFS) as qkp, \
             tc.tile_pool(name="vtsb", bufs=2) as vtsb, \
             tc.tile_pool(name="ps_mm", bufs=PSMM_BUFS, space="PSUM") as ps_mm:
            w_qk_f8 = [wqk_pool.tile([128, KK, 512], FP8, name=f"wqk{p}")
                       for p in range(3)]
            w_vT = wqk_pool.tile([128, KK, C], BF16)   # holds 64*w_v
            vstrips = _vt_strips()
            state = {"vt_sb": None, "qk_pair": []}

            def xstage(t):
                """x load + bf16 pre-cast + PE transpose + bf16/fp8 casts."""
                t0 = t * TOK_TILE
                x_t = xin.tile([128, CHUNKS, C], BF16, name="x_t")
                nc.gpsimd.dma_start(
                    x_t, x_d[t0:t0 + TOK_TILE, :].rearrange(
                        "(c p) f -> p c f", p=128))
                xT_bf = xtp.tile([128, KK, TOK_TILE], BF16, name="xT_bf")
                xT_f8 = xtp.tile([128, KK, TOK_TILE], FP8, name="xT_f8")
                for kk in range(KK):
                    xps = ps_tr.tile([128, TOK_TILE], BF16, name="xps",
                                     tag="tr")
                    for c in range(CHUNKS):
                        nc.tensor.transpose(
                            xps[:, c * 128:(c + 1) * 128],
                            x_t[:, c, kk * 128:(kk + 1) * 128], ident)
                    if kk % 2 == 0:
                        nc.vector.tensor_copy(xT_bf[:, kk, :], xps)
                        nc.scalar.copy(xT_f8[:, kk, :], xps)
                    else:
                        nc.scalar.copy(xT_bf[:, kk, :], xps)
                        nc.vector.tensor_copy(xT_f8[:, kk, :], xps)
                return xT_bf, xT_f8

            def mmstage(t, xT_bf, xT_f8):
                """qk + vT matmuls, vt strips, covariance for one tile."""
                t0 = t * TOK_TILE
                qk_t = qkp.tile([128, CHUNKS, 1536], FP8, name="qk_t")

                # qk = xT.T @ w_qk (token-major; fp8 DoubleRow pairs)
                for c in range(CHUNKS):
                    for p in range(3):
                        mmps = ps_mm.tile([128, 512], FP32, name="mmps",
                                          tag="s")
                        for i in range(KK // 2):
                            nc.tensor.matmul(
                                mmps,
                                xT_f8[:, 2 * i:2 * i + 2,
                                      c * 128:(c + 1) * 128],
                                w_qk_f8[p][:, 2 * i:2 * i + 2, :],
                                start=(i == 0), stop=(i == KK // 2 - 1),
                                perf_mode=DR)
                        if (c * 3 + p) % 2 == 0:
                            nc.scalar.mul(
                                qk_t[:, c, p * 512:(p + 1) * 512], mmps, C_QK)
                        else:
                            nc.vector.tensor_scalar_mul(
                                qk_t[:, c, p * 512:(p + 1) * 512], mmps, C_QK)

                # covariance + Gram over a pair of tiles: per head
                # [Gq | Gk | C] = [q'q | k'k | q'k], DoubleRow chunk pairs.
                # Emitted before vT on odd tiles so the final flush (and the
                # phase-2 chain it gates) overlaps the last tile's vT matmuls.
                state["qk_pair"].append(qk_t)
                if t % 2 == 1:
                    qk_pair = state["qk_pair"]
                    for h in range(H):
                        cg_ps = ps_mm.tile([HD, 288], FP32, name="cg_ps",
                                           tag="s")
                        np_ = 2 * len(qk_pair)
                        for i in range(np_):
                            qkx = qk_pair[i // 2]
                            lo = (i % 2) * 2
                            q_sl = qkx[:, lo:lo + 2, HD * h:HD * h + HD]
                            k_sl = qkx[:, lo:lo + 2,
                                       C + HD * h:C + HD * h + HD]
                            nc.tensor.matmul(
                                cg_ps[:, 0:HD], q_sl, q_sl,
                                start=(i == 0), stop=False, perf_mode=DR)
                            nc.tensor.matmul(
                                cg_ps[:, HD:2 * HD], k_sl, k_sl,
                                start=False, stop=False, perf_mode=DR)
                            nc.tensor.matmul(
                                cg_ps[:, 2 * HD:3 * HD], q_sl, k_sl,
                                start=False, stop=(i == np_ - 1),
                                perf_mode=DR)
                        nc.vector.tensor_add(
                            cg_accum[:, h, :], cg_ps, cg_accum[:, h, :])
                    state["qk_pair"] = []

            def vtstage(t, xT_bf, act_only=False):
                # vT = w_vT.T @ xT_bf (feature-major, c-major blocks);
                # batched over pairs of tiles to halve strip-DMA count
                if t % 2 == 0:
                    state["vt_sb"] = vtsb.tile([128, KK, 2 * TOK_TILE], BF16,
                                               name="vt_sb")
                vt_sb = state["vt_sb"]
                half = (t % 2) * TOK_TILE
                for m in range(KK):
                    vps = ps_mm.tile([128, TOK_TILE], FP32, name="vps",
                                     tag="s")
                    for kk in range(KK):
                        nc.tensor.matmul(
                            vps, w_vT[:, kk, m * 128:(m + 1) * 128],
                            xT_bf[:, kk, :],
                            start=(kk == 0), stop=(kk == KK - 1))
                    if act_only or m % 2 == 0:
                        nc.scalar.mul(
                            vt_sb[:, m, half:half + TOK_TILE], vps, 1.0 / S_W)
                    else:
                        nc.vector.tensor_scalar_mul(
                            vt_sb[:, m, half:half + TOK_TILE], vps, 1.0 / S_W)
                if t % 2 == 1:
                    tp0 = (t - 1) * TOK_TILE
                    for si, (m, p0, run, h, d0) in enumerate(vstrips):
                        src = vt_sb[p0:p0 + run, m, :]
                        dst = vt_all[d0:d0 + run, h, tp0:tp0 + 2 * TOK_TILE]
                        if si % 3 == 0:
                            nc.sync.dma_start(dst, src)
                        elif si % 3 == 1:
                            nc.scalar.dma_start(dst, src)
                        else:
                            nc.gpsimd.dma_start(dst, src)

            # w prep in groups of 4 row-blocks: one wide psum + one cast per
            # (group, kk) instead of 4 narrow ones, using the ps_mm banks
            # that sit idle until the first qk matmul. Interleaved with the
            # first two x stages: the first qk matmul only needs w_qk
            # p-block 0 (m 0..3) + xT(0).
            xT01 = [None, None]
            with tc.tile_pool(name="wload", bufs=2) as wload:
                def wprep(grp):
                    w_blk = wload.tile([128, 2, C], BF16, name="w_blk")
                    nc.gpsimd.dma_start(
                        w_blk,
                        wqkv_d[grp * 256:(grp + 1) * 256, :].rearrange(
                            "(b p) f -> p b f", p=128))
                    for kk in range(KK):
                        tps = ps_mm.tile([128, 256], BF16, name="wps",
                                         tag="s")
                        for b in range(2):
                            nc.tensor.transpose(
                                tps[:, b * 128:(b + 1) * 128],
                                w_blk[:, b, kk * 128:(kk + 1) * 128], ident)
                        if grp < 6:
                            dst = w_qk_f8[grp // 2][
                                :, kk, (grp % 2) * 256:(grp % 2) * 256 + 256]
                            if (grp + kk) % 2 == 0:
                                nc.vector.tensor_scalar_mul(dst, tps, S_W)
                            else:
                                nc.scalar.mul(dst, tps, S_W)
                        else:
                            base = (grp - 6) * 256
                            dst = w_vT[:, kk, base:base + 256]
                            if kk % 2 == 0:
                                nc.vector.tensor_scalar_mul(dst, tps, S_W)
                            else:
                                nc.scalar.mul(dst, tps, S_W)

                xT01[0] = xstage(0)
                wprep(0)
                wprep(1)
                xT01[1] = xstage(1)
                wprep(2)
                wprep(3)
                for grp in range(4, 9):
                    wprep(grp)

            for t in range(N_TILES):
                xts = xT01[t] if t < 2 else state.pop(("x", t))
                mmstage(t, *xts)
                if t < 6:
                    vtstage(t, xts[0])
                else:
                    state[("xts", t)] = xts
                if t + 2 < N_TILES:
                    state[("x", t + 2)] = xstage(t + 2)

            phase2(nc, tc, singles, dram, ps_tr, cg_accum, attnT, temp_all,
                   ident96f)

            # deferred vT for the last tile pair: fills the PE while the
            # phase-2 DVE/ACT softmax chain runs
            vtstage(6, state.pop(("xts", 6))[0])
            vtstage(7, state.pop(("xts", 7))[0])

        # ---------------- phase 3 pools; w_projT prep emitted first so the
        # PE has work while the DVE/ACT-heavy phase 2 chain runs ----------
        with tc.tile_pool(name="wpp", bufs=1) as wpp, \
             tc.tile_pool(name="wpload", bufs=2) as wpload, \
             tc.tile_pool(name="ot96p", bufs=2) as ot96p, \
             tc.tile_pool(name="otp", bufs=2) as otp, \
             tc.tile_pool(name="yp", bufs=2) as yp, \
             tc.tile_pool(name="ps_o", bufs=3, space="PSUM") as ps_o, \
             tc.tile_pool(name="ps_y", bufs=3, space="PSUM") as ps_y:
            # w_proj (cout, c) -> w_projT128 [128, m, cout] (dense c-major)
            w_projT = wpp.tile([128, KK, C], BF16)
            for n in range(KK):
                wp_blk = wpload.tile([128, C], BF16, name="wp_blk")
                nc.gpsimd.dma_start(wp_blk, wproj_d[n * 128:(n + 1) * 128, :])
                for m in range(KK):
                    tps2 = ps_tr.tile([128, 128], BF16, name="tps2", tag="tr")
                    nc.tensor.transpose(
                        tps2, wp_blk[:, m * 128:(m + 1) * 128], ident)
                    if (n + m) % 2 == 0:
                        nc.vector.tensor_copy(
                            w_projT[:, m, n * 128:(n + 1) * 128], tps2)
                    else:
                        nc.scalar.copy(
                            w_projT[:, m, n * 128:(n + 1) * 128], tps2)

            # ---------------- phase 3: attn@v + proj, sw-pipelined over
            # 2-tile groups (halves the relayout strip-DMA count) ----------
            ostrips = _ot_strips()
            T2 = 2 * TOK_TILE

            def attnv_group(g):
                g0 = g * T2
                ot96 = ot96p.tile([HD, H, T2], BF16, name="ot96")
                otsb = otp.tile([128, KK, T2], BF16, name="otsb")
                for half in range(2):
                    t0 = g0 + half * TOK_TILE
                    for h in range(H):
                        ops_ = ps_o.tile([HD, TOK_TILE], FP32, name="ops_")
                        nc.tensor.matmul(ops_, attnT[:, h, :],
                                         vt_all[:, h, t0:t0 + TOK_TILE],
                                         start=True, stop=True)
                        hh0 = half * TOK_TILE
                        if h % 2 == 0:
                            nc.vector.tensor_copy(
                                ot96[:, h, hh0:hh0 + TOK_TILE], ops_)
                        else:
                            nc.scalar.copy(
                                ot96[:, h, hh0:hh0 + TOK_TILE], ops_)
                for si, (h, d0, run, m, p0) in enumerate(ostrips):
                    src = ot96[d0:d0 + run, h, :]
                    dst = otsb[p0:p0 + run, m, :]
                    if si % 3 == 0:
                        nc.sync.dma_start(dst, src)
                    elif si % 3 == 1:
                        nc.scalar.dma_start(dst, src)
                    else:
                        nc.gpsimd.dma_start(dst, src)
                return otsb

            def proj_group(g, otsb):
                for piece in range(CHUNKS):
                    t0 = g * T2 + piece * 256
                    y_t = yp.tile([128, 2, C], FP32, name="y_t")
                    for c in range(2):
                        cc = piece * 2 + c
                        for (off, width) in ((0, 512), (512, 256)):
                            yps = ps_y.tile([128, 512], FP32, name="yps")
                            for m in range(KK):
                                nc.tensor.matmul(
                                    yps[:, :width],
                                    otsb[:, m, cc * 128:(cc + 1) * 128],
                                    w_projT[:, m, off:off + width],
                                    start=(m == 0), stop=(m == KK - 1))
                            nc.vector.tensor_tensor(
                                y_t[:, c, off:off + width], yps[:, :width],
                                b_all[:, off:off + width], mybir.AluOpType.add)
                    nc.scalar.dma_start(
                        out_d[t0:t0 + 256, :].rearrange(
                            "(c p) f -> p c f", p=128),
                        y_t)

            pend = None
            for g in range(N_TILES // 2):
                cur = attnv_group(g)
                if pend is not None:
                    proj_group(*pend)
                pend = (g, cur)
            proj_group(*pend)


def _get_nc():
    global _CACHED_NC
    if _CACHED_NC is None:
        _CACHED_NC = build_nc()
    return _CACHED_NC


def kernel(x, w_qkv, temperature, w_proj, b_proj):
    nc = _get_nc()
    x = np.ascontiguousarray(np.asarray(x, dtype=np.float32))
    in_maps = []
    for b in range(8):
        in_maps.append({
            "x": x[b],
            "w_qkv": np.asarray(w_qkv, dtype=np.float32),
            "temperature": np.asarray(temperature, dtype=np.float32),
            "w_proj": np.asarray(w_proj, dtype=np.float32),
            "b_proj": np.asarray(b_proj, dtype=np.float32),
        })
    res = run_bass_kernel_spmd(nc, in_maps, core_ids=list(range(8)))
    return np.stack([r["out"] for r in res.results], axis=0)
